# revision 1
# baseline (speedup 1.0000x reference)
"""Trainium2 Bass kernel for a pre-norm transformer block (nn_Block_74766790689102).

Strategy (8 NeuronCores, zero-communication SPMD):
  core c handles batch b=c//4, query chunk q=c%4 (512 of 2048 tokens).
  Each core redundantly computes K/V for its whole batch (attention needs all
  keys), and everything else (qkv for its chunk, attention, proj, MLP) only for
  its own 512-token chunk. Inputs are host-rotated so every core's chunk sits at
  token positions 0:512 -> one identical SPMD program for all 8 cores.

  Numerics: float32r (fp32 storage, fast PE mode, ~1e-4 rms rounding) for all
  projection/MLP matmuls; bf16 for attention score/PV matmuls. LayerNorm gains
  are folded into weights on the host; LN is applied as (x-mu)*rstd via
  PE-broadcast of per-token stats in feature-major layout.
"""

import os
import sys
import types

import numpy as np

DIM = 1024
HEADS = 16
HD = 64
HIDDEN = 4096
T = 2048          # tokens per batch
CH = 512          # chunk tokens per core
SCALE = HD ** -0.5
EPS = 1e-5
NCT = DIM // 128  # 8 c-tiles
NTC = T // 512    # 4 token chunks
NTT = T // 128    # 16 token tiles
P = 128

_ENV_READY = False
_PROG = None


def _setup_env():
    global _ENV_READY
    if _ENV_READY:
        return
    if "/opt/trn_rl_repo" not in sys.path:
        sys.path.insert(0, "/opt/trn_rl_repo")
    # NTFF profile hook shim (the RL container's antenv lacks axon_hooks).
    try:
        import antenv
        if "antenv.axon_hooks" not in sys.modules:
            mod = types.ModuleType("antenv.axon_hooks")
            mod._hook = None
            mod.set_axon_ntff_profile_hook = lambda h: setattr(mod, "_hook", h)
            mod.get_axon_ntff_profile_hook = lambda: mod._hook
            sys.modules["antenv.axon_hooks"] = mod
            antenv.axon_hooks = mod
        if os.environ.get("BASS_PROFILE"):
            from trn_agent_boot.trn_boot import _ntff_profile_via_ctypes
            sys.modules["antenv.axon_hooks"].set_axon_ntff_profile_hook(
                _ntff_profile_via_ctypes("/opt/axon/libaxon_pjrt.so"))
    except Exception:
        pass
    _ENV_READY = True


def _build_program():
    """Build + compile the single-core Bass program (same for all 8 cores)."""
    _setup_env()
    import concourse.bacc as bacc
    import concourse.tile as tile
    import concourse.mybir as mybir
    from concourse.masks import make_identity

    dt = mybir.dt
    AF = mybir.ActivationFunctionType
    ALU = mybir.AluOpType

    nc = bacc.Bacc("TRN2", target_bir_lowering=False, debug=False, num_devices=8)

    # ---- I/O ----
    xT_d = nc.dram_tensor("xT_d", [DIM, T], dt.float32, kind="ExternalInput").ap()
    xtm_d = nc.dram_tensor("xtm_d", [T, DIM], dt.float32, kind="ExternalInput").ap()
    wkq_d = nc.dram_tensor("wkq_d", [NCT, P, NCT, 256], dt.float32, kind="ExternalInput").ap()
    wvT_d = nc.dram_tensor("wvT_d", [DIM, DIM], dt.float32, kind="ExternalInput").ap()
    bq64_d = nc.dram_tensor("bq64_d", [P, NCT], dt.float32, kind="ExternalInput").ap()
    bk_d = nc.dram_tensor("bk_d", [P, NCT], dt.float32, kind="ExternalInput").ap()
    bvbc_d = nc.dram_tensor("bvbc_d", [P, DIM], dt.float32, kind="ExternalInput").ap()
    wp_d = nc.dram_tensor("wp_d", [P, NCT, DIM], dt.float32, kind="ExternalInput").ap()
    bpbc_d = nc.dram_tensor("bpbc_d", [P, DIM], dt.float32, kind="ExternalInput").ap()
    w1h_d = nc.dram_tensor("w1h_d", [HIDDEN // P, P, NCT, P], dt.float32, kind="ExternalInput").ap()
    b1pp_d = nc.dram_tensor("b1pp_d", [P, HIDDEN // P], dt.float32, kind="ExternalInput").ap()
    w2T_d = nc.dram_tensor("w2T_d", [HIDDEN, DIM], dt.float32, kind="ExternalInput").ap()
    b2bc_d = nc.dram_tensor("b2bc_d", [P, DIM], dt.float32, kind="ExternalInput").ap()
    out_d = nc.dram_tensor("out_d", [CH, DIM], dt.float32, kind="ExternalOutput").ap()

    f32, f32r, bf16 = dt.float32, dt.float32r, dt.bfloat16

    with tile.TileContext(nc) as tc:
        with tc.tile_pool(name="const", bufs=1) as cst, \
             tc.tile_pool(name="dram", bufs=1, space="DRAM") as dpool:

            k_dram = dpool.tile([NCT, P, NTC, 512], bf16)      # K feature-major, per pair
            o_dram = dpool.tile([64, HEADS, 512], dt.float32)  # attention out (feature-major)
            res1_dram = dpool.tile([CH, DIM], dt.float32)      # attn residual stream

            idf = cst.tile([P, P], f32)
            make_identity(nc, idf[:])
            idr = cst.tile([P, P], f32r)
            nc.vector.tensor_copy(idr[:], idf[:])
            ones_f = cst.tile([1, P], f32)
            nc.vector.memset(ones_f[:], 1.0)
            ones128_r = cst.tile([1, P], f32r)
            nc.vector.tensor_copy(ones128_r[:], ones_f[:])

            bq64 = cst.tile([P, NCT], f32)
            nc.sync.dma_start(bq64[:], bq64_d[:])
            bk_pp = cst.tile([P, NCT], f32)
            nc.sync.dma_start(bk_pp[:], bk_d[:])
            b1_pp = cst.tile([P, HIDDEN // P], f32)
            nc.sync.dma_start(b1_pp[:], b1pp_d[:])

            eps_t = cst.tile([P, 1], f32)
            nc.vector.memset(eps_t[:], EPS)

            # ---------------- Phase 1+2: LN1 stats, normalize, QKV ----------------
            vsbp_cm = tc.tile_pool(name="vsbp", bufs=1)
            vsp = vsbp_cm.__enter__()
            vsb = vsp.tile([P, NTT, HEADS, 128], bf16)     # V-hat token-major (64KB/part)
            xnp_cm = tc.tile_pool(name="xnp", bufs=1)
            xnp = xnp_cm.__enter__()
            xnT = xnp.tile([P, NCT, T], f32r)              # normalized x^T (64KB/part)

            with                  tc.tile_pool(name="ph12", bufs=1) as p12, \
                 tc.tile_pool(name="st", bufs=2) as stp, \
                 tc.tile_pool(name="wres", bufs=1) as wrp, \
                 tc.tile_pool(name="npsum", bufs=2, space="PSUM") as nps, \
                 tc.tile_pool(name="vpsum", bufs=3, space="PSUM") as vps:

                bv_bc = p12.tile([P, DIM], f32, tag="bvbc", bufs=1)
                nc.sync.dma_start(bv_bc[:], bvbc_d[:])
                onesc_f = p12.tile([P, 256], f32, tag="ocf", bufs=1)
                nc.vector.memset(onesc_f[:], 1.0)
                nc.vector.tensor_copy(
                    vsb[:, :, :, 0:1].rearrange("p a b c -> p (a b c)"), onesc_f[:])

                # LN1 stats + normalize + V-hat, fused per 512-token chunk
                wv_sb = wrp.tile([P, NCT, DIM], f32r)
                nc.sync.dma_start(wv_sb[:], wvT_d.rearrange("(a p) o -> p a o", p=P).bitcast(f32r))
                for tcn in range(NTC):
                    muc = stp.tile([1, 512], f32r, tag="muc", bufs=2, name=f"muc{tcn}")
                    rsc = stp.tile([1, 512], f32r, tag="rsc", bufs=2, name=f"rsc{tcn}")
                    for si in range(4):
                        s = tcn * 4 + si
                        xs = p12.tile([P, DIM], f32, tag="xs", bufs=2)
                        nc.sync.dma_start(xs[:], xtm_d[s * P:(s + 1) * P, :])
                        stats = stp.tile([P, 2, 6], f32, tag="bst")
                        for g in range(2):
                            nc.vector.bn_stats(stats[:, g, :], xs[:, g * 512:(g + 1) * 512])
                        mv = stp.tile([P, 2], f32, tag="mv")
                        nc.vector.bn_aggr(mv[:], stats[:])
                        sdv = stp.tile([P, 1], f32, tag="sdv")
                        nc.scalar.activation(sdv[:], mv[:, 1:2], AF.Sqrt, bias=eps_t[:])
                        rs = stp.tile([P, 1], f32, tag="rs")
                        nc.vector.reciprocal(rs[:], sdv[:])
                        pmu = nps.tile([1, P], f32, tag="nps")
                        nc.tensor.transpose(pmu[:], mv[:, 0:1], idf[:])
                        nc.vector.tensor_copy(muc[0:1, si * P:(si + 1) * P], pmu[:])
                        prs = nps.tile([1, P], f32, tag="nps")
                        nc.tensor.transpose(prs[:], rs[:], idf[:])
                        nc.vector.tensor_copy(rsc[0:1, si * P:(si + 1) * P], prs[:])
                    sl = slice(tcn * 512, (tcn + 1) * 512)
                    mb = nps.tile([P, 512], f32, tag="nps")
                    nc.tensor.matmul(mb[:], ones128_r[:], muc[:], start=True, stop=True)
                    rb = nps.tile([P, 512], f32, tag="nps")
                    nc.tensor.matmul(rb[:], ones128_r[:], rsc[:], start=True, stop=True)
                    mb_sb = p12.tile([P, 512], f32, tag="mbsb", bufs=2)
                    nc.scalar.activation(mb_sb[:], mb[:], AF.Copy)
                    rb_sb = p12.tile([P, 512], f32, tag="rbsb", bufs=2)
                    nc.scalar.activation(rb_sb[:], rb[:], AF.Copy)
                    for ct in range(NCT):
                        xt = p12.tile([P, 512], f32, tag="xt", bufs=2)
                        nc.sync.dma_start(xt[:], xT_d[ct * P:(ct + 1) * P, sl])
                        tmp = p12.tile([P, 512], f32, tag="tmp", bufs=2)
                        nc.gpsimd.tensor_tensor(tmp[:], xt[:], mb_sb[:], ALU.subtract)
                        nc.vector.tensor_tensor(xnT[:, ct, sl], tmp[:], rb_sb[:], ALU.mult)
                    # V-hat for this chunk's four token tiles
                    for ti in range(4):
                        tt = tcn * 4 + ti
                        for oc in range(2):
                            vp = vps.tile([P, 512], f32, tag="vqp")
                            for k in range(NCT):
                                nc.tensor.matmul(vp[:], xnT[:, k, tt * P:(tt + 1) * P],
                                                 wv_sb[:, k, oc * 512:(oc + 1) * 512],
                                                 start=(k == 0), stop=(k == NCT - 1))
                            nc.vector.tensor_tensor(
                                vsb[:, tt, oc * 8:(oc + 1) * 8, 64:128],
                                vp[:].rearrange("p (h d) -> p h d", d=64),
                                bv_bc[:, oc * 512:(oc + 1) * 512].rearrange(
                                    "p (h d) -> p h d", d=64),
                                ALU.add)

            # ------------- Phase 2b: K/Q production (K -> DRAM bf16, Q -> SBUF) -------------
            q_sb = vsp.tile([P, NCT, 512], bf16)
            with tc.tile_pool(name="kqp", bufs=2) as kqpool, \
                 tc.tile_pool(name="kqpsum", bufs=3, space="PSUM") as kqs:
                for j in range(NCT):
                    wkq = kqpool.tile([P, NCT, 256], f32r, tag="wkq", name=f"wkq{j}")
                    nc.sync.dma_start(wkq[:], wkq_d[j].bitcast(f32r))
                    for tcn in range(NTC):
                        kp = kqs.tile([P, 512], f32, tag="kq", name=f"kp{j}_{tcn}")
                        for k in range(NCT):
                            nc.tensor.matmul(kp[:], wkq[:, k, 0:P],
                                             xnT[:, k, tcn * 512:(tcn + 1) * 512],
                                             start=(k == 0), stop=(k == NCT - 1))
                        kev = kqpool.tile([P, 512], bf16, tag="kev", name=f"kev{j}_{tcn}")
                        nc.vector.tensor_scalar(kev[:], kp[:], bk_pp[:, j:j + 1],
                                                None, ALU.add)
                        nc.sync.dma_start(k_dram[j, :, tcn, :], kev[:])
                    qp = kqs.tile([P, 512], f32, tag="kq", name=f"qp{j}")
                    for k in range(NCT):
                        nc.tensor.matmul(qp[:], wkq[:, k, P:256], xnT[:, k, 0:512],
                                         start=(k == 0), stop=(k == NCT - 1))
                    nc.vector.tensor_scalar(q_sb[:, j, :], qp[:], bq64[:, j:j + 1],
                                            None, ALU.add)

            xnp_cm.__exit__(None, None, None)

            # ------------- Phase 3: attention (pair-merged exp, scores(j)/PV(j-1) pipeline) -------------
            with tc.tile_pool(name="att", bufs=2) as att, \
                 tc.tile_pool(name="pst", bufs=2) as pstp, \
                 tc.tile_pool(name="spsum", bufs=2, space="PSUM") as sps, \
                 tc.tile_pool(name="pvpsum", bufs=3, space="PSUM") as pvs, \
                 tc.tile_pool(name="bcpsum", bufs=1, space="PSUM") as bcs:
                psbs = {}

                def tail(h, pv):
                    dn = att.tile([1, 512], f32, tag="dn", name=f"dn{h}")
                    nc.vector.tensor_copy(dn[:], pv[0:1, :])
                    rcp = att.tile([1, 512], f32, tag="rcp", name=f"rcp{h}")
                    nc.vector.reciprocal(rcp[:], dn[:])
                    rcr = att.tile([1, 512], f32r, tag="rcr", name=f"rcr{h}")
                    nc.vector.tensor_copy(rcr[:], rcp[:])
                    bc = bcs.tile([P, 512], f32, tag="bc", name=f"bcp{h}")
                    nc.tensor.matmul(bc[:], ones128_r[:], rcr[:], start=True, stop=True)
                    bc_sb = att.tile([P, 512], f32, tag="bcsb", name=f"bsb{h}")
                    nc.vector.tensor_copy(bc_sb[64:128, :], bc[64:128, :])
                    o_loc = att.tile([P, 512], f32, tag="oloc", name=f"ol{h}")
                    nc.vector.tensor_tensor(o_loc[64:96, :], pv[64:96, :],
                                            bc_sb[64:96, :], ALU.mult)
                    nc.vector.tensor_tensor(o_loc[96:128, :], pv[96:128, :],
                                            bc_sb[96:128, :], ALU.mult)
                    nc.sync.dma_start(o_dram[:, h, :], o_loc[64:128, :])

                def emit_scores(j):
                    kin = att.tile([P, T], bf16, tag="kin", name=f"kin{j}")
                    nc.sync.dma_start(kin[:], k_dram[j, :, :, :])
                    psb = pstp.tile([P, NTT, 2, 512], bf16, tag="p", name=f"psb{j}")
                    psbs[j] = psb
                    for kt in range(NTT):
                        sp = sps.tile([P, 2, 512], f32, tag="s", name=f"sp{j}_{kt}")
                        nc.tensor.matmul(sp[:, 0, :], kin[0:64, kt * P:(kt + 1) * P],
                                         q_sb[0:64, j, :], start=True, stop=True)
                        nc.tensor.matmul(sp[:, 1, :], kin[64:128, kt * P:(kt + 1) * P],
                                         q_sb[64:128, j, :], start=True, stop=True)
                        nc.scalar.activation(psb[:, kt, :, :], sp[:], AF.Exp, scale=SCALE)

                def emit_pv(j):
                    psb = psbs.pop(j)
                    for hh in range(2):
                        h = 2 * j + hh
                        pv = pvs.tile([P, 512], f32, tag="pv", name=f"pv{h}")
                        for kt in range(NTT):
                            nc.tensor.matmul(pv[:], vsb[:, kt, h, :], psb[:, kt, hh, :],
                                             start=(kt == 0), stop=(kt == NTT - 1))
                        tail(h, pv)

                emit_scores(0)
                for j in range(1, NCT):
                    emit_scores(j)
                    emit_pv(j - 1)
                emit_pv(NCT - 1)

            vsbp_cm.__exit__(None, None, None)

            # ---------------- Phase 4: proj + residual (-> res1_dram) ----------------
            with tc.tile_pool(name="prj", bufs=2) as prj, \
                 tc.tile_pool(name="wpp", bufs=1) as wpp, \
                 tc.tile_pool(name="pjps", bufs=2, space="PSUM") as pjs:
                wp_sb = wpp.tile([P, NCT, DIM], f32r)
                for j in range(NCT):
                    nc.sync.dma_start(wp_sb[:, j, :], wp_d[:, j, :].bitcast(f32r))
                bp_bc = wpp.tile([P, DIM], f32)
                nc.sync.dma_start(bp_bc[:], bpbc_d[:])
                osb = wpp.tile([P, NCT, 512], f32r)
                nc.sync.dma_start(osb[0:64, :, :],
                                  o_dram[:, 0::2, :].bitcast(f32r))
                nc.sync.dma_start(osb[64:128, :, :],
                                  o_dram[:, 1::2, :].bitcast(f32r))
                for ts in range(4):
                    xres = prj.tile([P, DIM], f32, tag="xres")
                    nc.sync.dma_start(xres[:], xtm_d[ts * P:(ts + 1) * P, :])
                    for oc in range(2):
                        pj = pjs.tile([P, 512], f32, tag="pj")
                        for j in range(NCT):
                            nc.tensor.matmul(pj[:], osb[:, j, ts * P:(ts + 1) * P],
                                             wp_sb[:, j, oc * 512:(oc + 1) * 512],
                                             start=(j == 0), stop=(j == NCT - 1))
                        t1 = prj.tile([P, 512], f32, tag="t1")
                        nc.vector.tensor_tensor(t1[:], pj[:],
                                                bp_bc[:, oc * 512:(oc + 1) * 512], ALU.add)
                        r1 = prj.tile([P, 512], f32, tag="r1")
                        nc.vector.tensor_tensor(r1[:], t1[:],
                                                xres[:, oc * 512:(oc + 1) * 512], ALU.add)
                        nc.sync.dma_start(
                            res1_dram[ts * P:(ts + 1) * P, oc * 512:(oc + 1) * 512], r1[:])

            # ---------------- Phase 5: MLP ----------------
            with tc.tile_pool(name="mlp", bufs=3) as mlp, \
                 tc.tile_pool(name="resp", bufs=1) as resp, \
                 tc.tile_pool(name="h3tp", bufs=1) as h3tp, \
                 tc.tile_pool(name="st2", bufs=2) as st2, \
                 tc.tile_pool(name="tps", bufs=1, space="PSUM") as tps, \
                 tc.tile_pool(name="f1ps", bufs=2, space="PSUM") as f1s, \
                 tc.tile_pool(name="f2ps", bufs=5, space="PSUM") as f2s:
                res1 = resp.tile([P, NTC, DIM], f32)
                nc.sync.dma_start(res1[:], res1_dram[:].rearrange("(a p) d -> p a d", p=P))
                h2t = resp.tile([P, NCT, 512], f32r)
                h3t = h3tp.tile([P, HIDDEN // P, 512], f32r)
                b2_bc = resp.tile([P, DIM], f32)
                nc.sync.dma_start(b2_bc[:], b2bc_d[:])

                # LN2 (token-major stats, fused apply) + transpose to feature-major
                for ts in range(4):
                    stats = st2.tile([P, 2, 6], f32, tag="bst2")
                    for g in range(2):
                        nc.vector.bn_stats(stats[:, g, :], res1[:, ts, g * 512:(g + 1) * 512])
                    mv = st2.tile([P, 2], f32, tag="mv2")
                    nc.vector.bn_aggr(mv[:], stats[:])
                    sdv = st2.tile([P, 1], f32, tag="sdv2")
                    nc.scalar.activation(sdv[:], mv[:, 1:2], AF.Sqrt, bias=eps_t[:])
                    rs2 = st2.tile([P, 1], f32, tag="rs2")
                    nc.vector.reciprocal(rs2[:], sdv[:])
                    nmu = st2.tile([P, 1], f32, tag="nmu")
                    nc.vector.tensor_tensor(nmu[:], mv[:, 0:1], rs2[:], ALU.mult)
                    nc.vector.tensor_scalar(nmu[:], nmu[:], -1.0, None, ALU.mult)
                    h2 = mlp.tile([P, DIM], f32r, tag="h2", bufs=2)
                    nc.scalar.activation(h2[:], res1[:, ts, :], AF.Identity,
                                         bias=nmu[:], scale=rs2[:])
                    for ct in range(NCT):
                        tp = tps.tile([P, P], f32r, tag="tp2")
                        nc.tensor.transpose(tp[:], h2[:, ct * P:(ct + 1) * P], idr[:])
                        nc.vector.tensor_copy(h2t[:, ct, ts * P:(ts + 1) * P], tp[:])

                # fc1 + gelu -> h3t (feature-major)
                for ot in range(HIDDEN // P):
                    w1c = mlp.tile([P, NCT, P], f32r, tag="w1c", bufs=3)
                    nc.sync.dma_start(w1c[:], w1h_d[ot].bitcast(f32r))
                    fp = f1s.tile([P, 512], f32, tag="f1")
                    for ct in range(NCT):
                        nc.tensor.matmul(fp[:], w1c[:, ct, :], h2t[:, ct, :],
                                         start=(ct == 0), stop=(ct == NCT - 1))
                    nc.scalar.activation(h3t[:, ot, :], fp[:], AF.Gelu,
                                         bias=b1_pp[:, ot:ot + 1])

                # fc2 + bias + residual -> out
                for oc in range(2):
                    f2t = [f2s.tile([P, 512], f32, tag="f2", name=f"f2_{oc}_{i}")
                           for i in range(4)]
                    for ct in range(HIDDEN // P):
                        w2t = mlp.tile([P, 512], f32r, tag="w2t")
                        nc.sync.dma_start(
                            w2t[:], w2T_d[ct * P:(ct + 1) * P, oc * 512:(oc + 1) * 512]
                            .bitcast(f32r))
                        for ts in range(4):
                            nc.tensor.matmul(f2t[ts][:], h3t[:, ct, ts * P:(ts + 1) * P],
                                             w2t[:], start=(ct == 0),
                                             stop=(ct == HIDDEN // P - 1))
                    for ts in range(4):
                        t1 = mlp.tile([P, 512], f32, tag="t12")
                        nc.vector.tensor_tensor(t1[:], f2t[ts][:],
                                                b2_bc[:, oc * 512:(oc + 1) * 512], ALU.add)
                        t2 = mlp.tile([P, 512], f32, tag="t22")
                        nc.vector.tensor_tensor(t2[:], t1[:],
                                                res1[:, ts, oc * 512:(oc + 1) * 512], ALU.add)
                        nc.sync.dma_start(
                            out_d[ts * P:(ts + 1) * P, oc * 512:(oc + 1) * 512], t2[:])

    nc.compile()
    return nc


def _get_program():
    global _PROG
    if _PROG is None:
        _PROG = _build_program()
    return _PROG


def _pack_cols(wT):
    """[C, O] -> [O//128, 128(p), C//128(k), 128(o)] so each o-tile DMA is contiguous."""
    C, O = wT.shape
    # out[ot, p, k, o] = wT[k*128+p, ot*128+o]
    return np.ascontiguousarray(
        wT.reshape(C // P, P, O // P, P).transpose(2, 1, 0, 3))


def _pack_wkq(wqT, wkT):
    """Combine K and Q o-tile packs: [8, 128, 8, 256] (K cols then Q cols)."""
    k = _pack_cols(wkT)
    q = _pack_cols(wqT)
    return np.ascontiguousarray(np.concatenate([k, q], axis=3))


def _host_prep(x, ln1_g, ln1_b, w_qkv, w_proj, b_proj, ln2_g, ln2_b,
               w_fc1, b_fc1, w_fc2, b_fc2):
    """Per-core input dicts. Pure layout/weight-folding work (no activation math)."""
    f = np.float32
    x = np.asarray(x, f)
    g1 = np.asarray(ln1_g, f); b1 = np.asarray(ln1_b, f)
    g2 = np.asarray(ln2_g, f); b2 = np.asarray(ln2_b, f)
    w_qkv = np.asarray(w_qkv, f); w_proj = np.asarray(w_proj, f)
    w_fc1 = np.asarray(w_fc1, f); w_fc2 = np.asarray(w_fc2, f)
    b_proj = np.asarray(b_proj, f); b_fc1 = np.asarray(b_fc1, f)
    b_fc2 = np.asarray(b_fc2, f)

    wq, wk, wv = w_qkv[0:DIM], w_qkv[DIM:2 * DIM], w_qkv[2 * DIM:3 * DIM]
    shared = {
        "wkq_d": _pack_wkq((wq * g1[None, :]).T, (wk * g1[None, :]).T),
        "wvT_d": np.ascontiguousarray((wv * g1[None, :]).T),
        "bq64_d": np.ascontiguousarray((wq @ b1).reshape(NCT, P).T),
        "bk_d": np.ascontiguousarray((wk @ b1).reshape(NCT, P).T),
        "bvbc_d": np.ascontiguousarray(np.broadcast_to(wv @ b1, (P, DIM))),
        "wp_d": np.ascontiguousarray(w_proj.T.reshape(NCT, P, DIM).transpose(1, 0, 2)),
        "bpbc_d": np.ascontiguousarray(np.broadcast_to(b_proj, (P, DIM))),
        "w1h_d": _pack_cols((w_fc1 * g2[None, :]).T),
        "b1pp_d": np.ascontiguousarray((b_fc1 + w_fc1 @ b2).reshape(HIDDEN // P, P).T),
        "w2T_d": np.ascontiguousarray(w_fc2.T),
        "b2bc_d": np.ascontiguousarray(np.broadcast_to(b_fc2, (P, DIM))),
    }
    in_maps = []
    for core in range(8):
        b, q = core // 4, core % 4
        xroll = np.roll(x[b], -CH * q, axis=0)
        m = dict(shared)
        m["xT_d"] = np.ascontiguousarray(xroll.T)
        m["xtm_d"] = np.ascontiguousarray(xroll)
        in_maps.append(m)
    return in_maps


def kernel(**inputs) -> np.ndarray:
    _setup_env()
    from concourse import bass_utils

    nc = _get_program()
    in_maps = _host_prep(**inputs)
    run_kwargs = {}
    if os.environ.get("BASS_PROFILE"):
        import tempfile
        run_kwargs = dict(trace=True, tmpdir=tempfile.mkdtemp(prefix="blk_prof"))
    res = bass_utils.run_bass_kernel_spmd(nc, in_maps, core_ids=list(range(8)),
                                          **run_kwargs)
    kernel.last_result = res
    x = np.asarray(inputs["x"])
    out = np.empty((2, T, DIM), np.float32)
    for core in range(8):
        b, q = core // 4, core % 4
        out[b, CH * q:CH * (q + 1), :] = res.results[core]["out_d"]
    return out



# revision 9
# speedup vs baseline: 1.4581x; 1.4581x over previous
"""Trainium2 Bass kernel for a pre-norm transformer block (nn_Block_74766790689102).

v2 strategy (8 NeuronCores, zero-communication SPMD):
  core c handles batch b=c//4, query chunk q=c%4 (512 of 2048 tokens); inputs
  are host-rotated so each core's chunk sits at token positions 0:512 -> one
  identical SPMD program for all 8 cores. Each core redundantly computes K/V
  for its whole batch (attention needs all keys).

  Changes vs v1 (716us):
  - bf16 operands for every matmul (fp32 PSUM accumulation): halves weight
    DMA and enables fast weight loads on the PE.
  - Normalized x is never materialized. QKV matmuls consume raw x^T plus a
    rank-2 correction row ((-mu, sdv) x (col-sums, bias)) folded into the
    contraction, then a per-token rstd scale on the outputs. Kills the
    gpsimd/vector normalize traffic and one full 8MB x reload.
  - K, exp'd scores, attention outputs, res1 all stay in SBUF (no DRAM
    roundtrips).
  - K production for head-pair j+1, scores/exp for j, and PV for j-1 are
    interleaved in one PE stream so the ~134us of scalar-engine exp hides
    behind PE work.
  - Softmax denominators collect into one [16,512] tile; a single
    reciprocal_approx_fast + 8 PE broadcasts replace 16 serial [1,512] DVE
    reciprocals (was 52us).
  - V/proj biases fold into the proj bias on the host (softmax rows sum to 1).
  - w1 is fully resident in SBUF before FC1 starts; w2 streams as bf16.
"""

import os
import sys
import types

import numpy as np
import ml_dtypes

DIM = 1024
HEADS = 16
HD = 64
HIDDEN = 4096
T = 2048          # tokens per batch
CH = 512          # chunk tokens per core
SCALE = HD ** -0.5
EPS = 1e-5
NCT = DIM // 128  # 8 feature tiles
NTC = T // 512    # 4 token chunks
NTT = T // 128    # 16 token tiles
P = 128

_ENV_READY = False
_PROG = None


def _setup_env():
    global _ENV_READY
    if _ENV_READY:
        return
    if "/opt/trn_rl_repo" not in sys.path:
        sys.path.insert(0, "/opt/trn_rl_repo")
    # NTFF profile hook shim (the RL container's antenv lacks axon_hooks).
    try:
        import antenv
        if "antenv.axon_hooks" not in sys.modules:
            mod = types.ModuleType("antenv.axon_hooks")
            mod._hook = None
            mod.set_axon_ntff_profile_hook = lambda h: setattr(mod, "_hook", h)
            mod.get_axon_ntff_profile_hook = lambda: mod._hook
            sys.modules["antenv.axon_hooks"] = mod
            antenv.axon_hooks = mod
        if os.environ.get("BASS_PROFILE"):
            from trn_agent_boot.trn_boot import _ntff_profile_via_ctypes
            sys.modules["antenv.axon_hooks"].set_axon_ntff_profile_hook(
                _ntff_profile_via_ctypes("/opt/axon/libaxon_pjrt.so"))
    except Exception:
        pass
    _ENV_READY = True


def _build_program():
    """Build + compile the single-core Bass program (same for all 8 cores)."""
    _setup_env()
    import concourse.bacc as bacc
    import concourse.tile as tile
    import concourse.mybir as mybir
    from concourse.masks import make_identity

    dt = mybir.dt
    AF = mybir.ActivationFunctionType
    ALU = mybir.AluOpType
    f32, bf16 = dt.float32, dt.bfloat16

    nc = bacc.Bacc("TRN2", target_bir_lowering=False, debug=False, num_devices=8)

    # ---- I/O ----
    xtm_d = nc.dram_tensor("xtm_d", [T, DIM], f32, kind="ExternalInput").ap()
    xsb_d = nc.dram_tensor("xsb_d", [T, DIM], bf16, kind="ExternalInput").ap()
    xTb_d = nc.dram_tensor("xTb_d", [DIM, T], bf16, kind="ExternalInput").ap()
    wv_d = nc.dram_tensor("wv_d", [DIM, DIM], bf16, kind="ExternalInput").ap()
    wv1_d = nc.dram_tensor("wv1_d", [1, DIM], bf16, kind="ExternalInput").ap()
    wkq_d = nc.dram_tensor("wkq_d", [NCT, P, NCT, 256], bf16, kind="ExternalInput").ap()
    wkqc_d = nc.dram_tensor("wkqc_d", [2, NCT, 256], bf16, kind="ExternalInput").ap()
    selb_d = nc.dram_tensor("selb_d", [16, NCT, P], bf16, kind="ExternalInput").ap()
    wp_d = nc.dram_tensor("wp_d", [P, NCT, DIM], bf16, kind="ExternalInput").ap()
    bpbc_d = nc.dram_tensor("bpbc_d", [P, DIM], f32, kind="ExternalInput").ap()
    w1h_d = nc.dram_tensor("w1h_d", [HIDDEN // P, P, NCT, P], bf16, kind="ExternalInput").ap()
    b1pp_d = nc.dram_tensor("b1pp_d", [P, HIDDEN // P], f32, kind="ExternalInput").ap()
    w2T_d = nc.dram_tensor("w2T_d", [HIDDEN, DIM], bf16, kind="ExternalInput").ap()
    b2bc_d = nc.dram_tensor("b2bc_d", [P, DIM], f32, kind="ExternalInput").ap()
    out_d = nc.dram_tensor("out_d", [CH, DIM], f32, kind="ExternalOutput").ap()

    with tile.TileContext(nc) as tc:
        with tc.tile_pool(name="cst", bufs=1) as cst, \
             tc.tile_pool(name="resp", bufs=1) as resp:

            # ---------------- constants ----------------
            idf = cst.tile([P, P], f32)
            make_identity(nc, idf[:])
            idb = cst.tile([P, P], bf16)
            nc.vector.tensor_copy(idb[:], idf[:])
            ones1b = cst.tile([1, P], bf16)
            nc.vector.memset(ones1b[:], 1.0)
            eps_t = cst.tile([P, 1], f32)
            nc.vector.memset(eps_t[:], EPS)
            # head-pair selection matrices for the denominator broadcast
            sel = cst.tile([16, NCT, P], bf16)
            nc.sync.dma_start(sel[:], selb_d[:])
            corr2 = cst.tile([2, T], bf16)     # rows: -mu, sdv (per token)
            rsr = cst.tile([1, T], bf16)       # rstd row (per token)
            rs_cols = cst.tile([P, NTT], f32)  # rstd, token-partition layout
            dnall = cst.tile([16, 512], f32)   # softmax denominators per head
            rc_f = cst.tile([16, 512], f32)
            rc_bf = cst.tile([16, 512], bf16)
            b1_pp = cst.tile([P, HIDDEN // P], f32)
            nc.sync.dma_start(b1_pp[:], b1pp_d[:])

            res1 = resp.tile([P, NTC, DIM], f32)   # attn residual stream
            b2_bc = resp.tile([P, DIM], f32)
            nc.sync.dma_start(b2_bc[:], b2bc_d[:])

            # attention-lifetime SBUF (released after proj)
            atn = tc.alloc_tile_pool(name="atn", bufs=1)
            vsb = atn.tile([P, NTT, HEADS, 65], bf16)   # V-hat + ones col 64
            q_sb = atn.tile([P, NCT, 512], bf16)
            k_sb = atn.tile([P, NCT, T], bf16)
            osb = atn.tile([P, NCT, 512], bf16)         # per-pair attn out
            rb_sb = atn.tile([P, NTC, 512], bf16)       # rstd broadcast

            nc.vector.memset(
                vsb[:, :, :, 64:65].rearrange("p a b c -> p (a b c)"), 1.0)

            # raw x^T + K/Q weights (released before FC1 weight prefetch)
            xTp = tc.alloc_tile_pool(name="xTp", bufs=1, side="right")
            xT_sb = xTp.tile([P, NCT, T], bf16)
            for k in range(NCT):
                nc.sync.dma_start(xT_sb[:, k, :], xTb_d[k * P:(k + 1) * P, :])
            kqw = tc.alloc_tile_pool(name="kqw", bufs=2, side="right")
            wkqc_sb = kqw.tile([2, NCT, 256], bf16, tag="wkqc", bufs=1)
            nc.sync.dma_start(wkqc_sb[:], wkqc_d[:])

            # ---------------- Phase A: LN1 stats + V-hat ----------------
            with tc.tile_pool(name="xsp", bufs=2) as xsp, \
                 tc.tile_pool(name="stp", bufs=2) as stp, \
                 tc.tile_pool(name="wvp", bufs=1) as wvp, \
                 tc.tile_pool(name="aps", bufs=1, space="PSUM") as aps, \
                 tc.tile_pool(name="rbp", bufs=1, space="PSUM") as rbp, \
                 tc.tile_pool(name="vps", bufs=3, space="PSUM") as vps:
                wv_sb = wvp.tile([P, NCT, DIM], bf16)
                nc.sync.dma_start(wv_sb[:], wv_d.rearrange("(a p) o -> p a o", p=P))
                wv1_sb = wvp.tile([1, DIM], bf16)
                nc.sync.dma_start(wv1_sb[:], wv1_d[:])
                for s in range(NTT):
                    xs = xsp.tile([P, DIM], bf16, tag="xs")
                    nc.sync.dma_start(xs[:], xsb_d[s * P:(s + 1) * P, :])
                    stats = stp.tile([P, 2, 6], f32, tag="bst")
                    for g in range(2):
                        nc.vector.bn_stats(stats[:, g, :], xs[:, g * 512:(g + 1) * 512])
                    mv = stp.tile([P, 2], f32, tag="mv")
                    nc.vector.bn_aggr(mv[:], stats[:])
                    stk = stp.tile([P, 2], f32, tag="stk")
                    nc.vector.tensor_scalar(stk[:, 0:1], mv[:, 0:1], -1.0, None, ALU.mult)
                    nc.scalar.activation(stk[:, 1:2], mv[:, 1:2], AF.Sqrt, bias=eps_t[:])
                    nc.vector.reciprocal(rs_cols[:, s:s + 1], stk[:, 1:2])
                    pst = aps.tile([2, P], f32, tag="pst")
                    nc.tensor.transpose(pst[:], stk[:], idf[:])
                    nc.vector.tensor_copy(corr2[:, s * P:(s + 1) * P], pst[:])
                    pst1 = aps.tile([1, P], f32, tag="pst1")
                    nc.tensor.transpose(pst1[:], rs_cols[:, s:s + 1], idf[:])
                    nc.vector.tensor_copy(rsr[:, s * P:(s + 1) * P], pst1[:])
                    # V-hat for token tile s (raw x + rank-1 mu correction)
                    for oc in range(2):
                        vp = vps.tile([P, 512], f32, tag="vp")
                        for k in range(NCT):
                            nc.tensor.matmul(vp[:], xT_sb[:, k, s * P:(s + 1) * P],
                                             wv_sb[:, k, oc * 512:(oc + 1) * 512],
                                             start=(k == 0), stop=False)
                        nc.tensor.matmul(vp[:], corr2[0:1, s * P:(s + 1) * P],
                                         wv1_sb[0:1, oc * 512:(oc + 1) * 512],
                                         start=False, stop=True)
                        nc.vector.tensor_scalar(
                            vsb[:, s, oc * 8:(oc + 1) * 8, 0:64],
                            vp[:].rearrange("p (h d) -> p h d", d=64),
                            rs_cols[:, s:s + 1], None, ALU.mult)
                    if s % 4 == 3:
                        tcn = s // 4
                        rb = rbp.tile([P, 512], f32, tag="rb")
                        nc.tensor.matmul(rb[:], ones1b[:],
                                         rsr[0:1, tcn * 512:(tcn + 1) * 512],
                                         start=True, stop=True)
                        nc.vector.tensor_copy(rb_sb[:, tcn, :], rb[:])

            # ---------------- Phase B+C: K/Q + scores/exp + PV, one PE stream ----------------
            psb_slots = {}
            wkq_tiles = {}
            w1_pool = []   # filled mid-phase once xT frees

            with tc.tile_pool(name="psbp", bufs=14) as psbp, \
                 tc.tile_pool(name="tvec", bufs=2) as tvec, \
                 tc.tile_pool(name="spp", bufs=2, space="PSUM") as spp, \
                 tc.tile_pool(name="kqp", bufs=1, space="PSUM") as kqp, \
                 tc.tile_pool(name="pvp", bufs=3, space="PSUM") as pvp:

                def load_wkq(j):
                    w = kqw.tile([P, NCT, 256], bf16, tag="wkq", name=f"wkq{j}")
                    nc.sync.dma_start(w[:], wkq_d[j])
                    wkq_tiles[j] = w

                def emit_k_chunk(j, tcn):
                    w = wkq_tiles[j]
                    kp = kqp.tile([P, 512], f32, tag="kq", name=f"kp{j}_{tcn}")
                    sl = slice(tcn * 512, (tcn + 1) * 512)
                    for k in range(NCT):
                        nc.tensor.matmul(kp[:], w[:, k, 0:P], xT_sb[:, k, sl],
                                         start=(k == 0), stop=False)
                    nc.tensor.matmul(kp[:], wkqc_sb[:, j, 0:P], corr2[0:2, sl],
                                     start=False, stop=True)
                    nc.vector.tensor_tensor(k_sb[:, j, sl], kp[:], rb_sb[:, tcn, :],
                                            ALU.mult)

                def emit_q(j):
                    w = wkq_tiles.pop(j)
                    qp = kqp.tile([P, 512], f32, tag="kq", name=f"qp{j}")
                    for k in range(NCT):
                        nc.tensor.matmul(qp[:], w[:, k, P:256], xT_sb[:, k, 0:512],
                                         start=(k == 0), stop=False)
                    nc.tensor.matmul(qp[:], wkqc_sb[:, j, P:256], corr2[0:2, 0:512],
                                     start=False, stop=True)
                    nc.vector.tensor_tensor(q_sb[:, j, :], qp[:], rb_sb[:, 0, :],
                                            ALU.mult)

                def emit_s(j, kt):
                    sp = spp.tile([P, 2, 512], f32, tag="sp", name=f"sp{j}_{kt}")
                    ksl = slice(kt * P, (kt + 1) * P)
                    nc.tensor.matmul(sp[:, 0, :], k_sb[0:64, j, ksl],
                                     q_sb[0:64, j, :], start=True, stop=True)
                    nc.tensor.matmul(sp[:, 1, :], k_sb[64:128, j, ksl],
                                     q_sb[64:128, j, :], start=True, stop=True)
                    slot = psbp.tile([P, 2, 512], bf16, tag="psb",
                                     name=f"psb{j}_{kt}")
                    nc.scalar.activation(slot[:], sp[:], AF.Exp, scale=SCALE)
                    psb_slots[(j, kt)] = slot

                def emit_pv_kt(j, kt, pvA, pvB):
                    slot = psb_slots.pop((j, kt))
                    nc.tensor.matmul(pvA[:], vsb[:, kt, 2 * j, :], slot[:, 0, :],
                                     start=(kt == 0), stop=(kt == NTT - 1))
                    nc.tensor.matmul(pvB[:], vsb[:, kt, 2 * j + 1, :], slot[:, 1, :],
                                     start=(kt == 0), stop=(kt == NTT - 1))

                def emit_tail(h, pv):
                    dtmp = tvec.tile([65, 512], f32, tag="dtmp", name=f"dt{h}")
                    nc.scalar.activation(dtmp[64:65, :], pv[64:65, :], AF.Copy)
                    nc.sync.dma_start(dnall[h:h + 1, :], dtmp[64:65, :])
                    if h % 2 == 0:
                        nc.vector.tensor_copy(osb[0:64, h // 2, :], pv[0:64, :])
                    else:
                        ot = tvec.tile([64, 512], bf16, tag="ot", name=f"ot{h}")
                        nc.vector.tensor_copy(ot[:], pv[0:64, :])
                        nc.sync.dma_start(osb[64:128, h // 2, :], ot[:])

                load_wkq(0)
                load_wkq(1)
                for tcn in range(NTC):
                    emit_k_chunk(0, tcn)
                emit_q(0)

                pvA = pvB = None
                for j in range(NCT):
                    if j == NCT - 1:
                        # x^T and K/Q weights are dead: free them; prefetch
                        # proj weights and the first w1 slice under the last
                        # scores/PV round.
                        kqw.release()
                        xTp.release()
                        w1a = tc.alloc_tile_pool(name="w1a", bufs=1,
                                                 side="right")
                        w1_sb_a = w1a.tile([P, 8, NCT, P], bf16)
                        for og in range(2):
                            nc.sync.dma_start(
                                w1_sb_a[:, og * 4:(og + 1) * 4, :, :],
                                w1h_d[og * 4:(og + 1) * 4].rearrange(
                                    "a p b c -> p a b c"))
                        w1_pool.append((w1a, w1_sb_a))
                        wpx = tc.alloc_tile_pool(name="wpx", bufs=1,
                                                 side="right")
                        wp_sb = wpx.tile([P, NCT, DIM], bf16)
                        nc.sync.dma_start(wp_sb[:], wp_d[:])
                        bp_bc = wpx.tile([P, DIM], f32)
                        nc.sync.dma_start(bp_bc[:], bpbc_d[:])
                    if j > 0:
                        pvA = pvp.tile([65, 512], f32, tag="pv", name=f"pvA{j - 1}")
                        pvB = pvp.tile([65, 512], f32, tag="pv", name=f"pvB{j - 1}")
                    for kt in range(NTT):
                        emit_s(j, kt)
                        if j < NCT - 1:
                            if kt in (1, 5, 9, 13):
                                if kt == 1:
                                    if j + 2 < NCT:
                                        load_wkq(j + 2)
                                emit_k_chunk(j + 1, kt // 4)
                            elif kt == 14:
                                emit_q(j + 1)
                        if j > 0:
                            emit_pv_kt(j - 1, kt, pvA, pvB)
                    if j > 0:
                        emit_tail(2 * (j - 1), pvA)
                        emit_tail(2 * (j - 1) + 1, pvB)
                # last pair
                pvA = pvp.tile([65, 512], f32, tag="pv", name="pvA7")
                pvB = pvp.tile([65, 512], f32, tag="pv", name="pvB7")
                for kt in range(NTT):
                    emit_pv_kt(NCT - 1, kt, pvA, pvB)
                emit_tail(2 * (NCT - 1), pvA)
                emit_tail(2 * (NCT - 1) + 1, pvB)

            # ---------------- Phase D: normalize o, proj + residual ----------------
            with tc.tile_pool(name="dvec", bufs=2) as dvec, \
                 tc.tile_pool(name="xrp", bufs=2) as xrp, \
                 tc.tile_pool(name="bcp", bufs=2, space="PSUM") as bcp, \
                 tc.tile_pool(name="pjp", bufs=2, space="PSUM") as pjp:
                nc.vector.reciprocal_approx_fast(rc_f[:], dnall[:])
                nc.vector.tensor_copy(rc_bf[:], rc_f[:])
                for j in range(NCT):
                    bc = bcp.tile([P, 512], f32, tag="bc", name=f"bc{j}")
                    nc.tensor.matmul(bc[:], sel[:, j, :], rc_bf[:],
                                     start=True, stop=True)
                    rcb = dvec.tile([P, 512], bf16, tag="rcb", name=f"rcb{j}")
                    nc.vector.tensor_copy(rcb[:], bc[:])
                    nc.vector.tensor_tensor(osb[:, j, :], osb[:, j, :], rcb[:],
                                            ALU.mult)
                for ts in range(NTC):
                    xres = xrp.tile([P, DIM], f32, tag="xres")
                    nc.sync.dma_start(xres[:], xtm_d[ts * P:(ts + 1) * P, :])
                    for oc in range(2):
                        pj = pjp.tile([P, 512], f32, tag="pj")
                        for j in range(NCT):
                            nc.tensor.matmul(pj[:], osb[:, j, ts * P:(ts + 1) * P],
                                             wp_sb[:, j, oc * 512:(oc + 1) * 512],
                                             start=(j == 0), stop=(j == NCT - 1))
                        t1 = dvec.tile([P, 512], f32, tag="t1")
                        nc.vector.tensor_tensor(t1[:], pj[:],
                                                xres[:, oc * 512:(oc + 1) * 512],
                                                ALU.add)
                        nc.vector.tensor_tensor(
                            res1[:, ts, oc * 512:(oc + 1) * 512], t1[:],
                            bp_bc[:, oc * 512:(oc + 1) * 512], ALU.add)

            atn.release()
            wpx.release()

            # rest of w1 (space freed by attention buffers)
            w1b = tc.alloc_tile_pool(name="w1b", bufs=1)
            w1_sb_b = w1b.tile([P, 24, NCT, P], bf16)
            for og in range(6):
                nc.sync.dma_start(
                    w1_sb_b[:, og * 4:(og + 1) * 4, :, :],
                    w1h_d[8 + og * 4:8 + (og + 1) * 4].rearrange(
                        "a p b c -> p a b c"))

            # ---------------- Phase E: LN2 + MLP ----------------
            with tc.tile_pool(name="st2", bufs=2) as st2, \
                 tc.tile_pool(name="h2p", bufs=2) as h2p, \
                 tc.tile_pool(name="h2tp", bufs=1) as h2tp, \
                 tc.tile_pool(name="h3p", bufs=1) as h3p, \
                 tc.tile_pool(name="w2p", bufs=6) as w2p, \
                 tc.tile_pool(name="mlv", bufs=2) as mlv, \
                 tc.tile_pool(name="tps", bufs=2, space="PSUM") as tps, \
                 tc.tile_pool(name="f1s", bufs=2, space="PSUM") as f1s, \
                 tc.tile_pool(name="f2s", bufs=4, space="PSUM") as f2s:
                h2t = h2tp.tile([P, NCT, 512], bf16)
                h3t = h3p.tile([P, HIDDEN // P, 512], bf16)

                for ts in range(NTC):
                    stats2 = st2.tile([P, 2, 6], f32, tag="bst2")
                    for g in range(2):
                        nc.vector.bn_stats(stats2[:, g, :],
                                           res1[:, ts, g * 512:(g + 1) * 512])
                    mv2 = st2.tile([P, 2], f32, tag="mv2")
                    nc.vector.bn_aggr(mv2[:], stats2[:])
                    sdv2 = st2.tile([P, 1], f32, tag="sdv2")
                    nc.scalar.activation(sdv2[:], mv2[:, 1:2], AF.Sqrt, bias=eps_t[:])
                    rs2 = st2.tile([P, 1], f32, tag="rs2")
                    nc.vector.reciprocal(rs2[:], sdv2[:])
                    nmu = st2.tile([P, 1], f32, tag="nmu")
                    nc.vector.tensor_tensor(nmu[:], mv2[:, 0:1], rs2[:], ALU.mult)
                    nc.vector.tensor_scalar(nmu[:], nmu[:], -1.0, None, ALU.mult)
                    h2 = h2p.tile([P, DIM], bf16, tag="h2")
                    nc.scalar.activation(h2[:], res1[:, ts, :], AF.Identity,
                                         bias=nmu[:], scale=rs2[:])
                    for ct in range(NCT):
                        tp = tps.tile([P, P], bf16, tag="tp")
                        nc.tensor.transpose(tp[:], h2[:, ct * P:(ct + 1) * P], idb[:])
                        nc.vector.tensor_copy(h2t[:, ct, ts * P:(ts + 1) * P], tp[:])

                # fc1 + gelu -> h3t (hidden-major)
                for ot in range(HIDDEN // P):
                    w1_sb = w1_pool[0][1] if ot < 8 else w1_sb_b
                    oti = ot if ot < 8 else ot - 8
                    fp = f1s.tile([P, 512], f32, tag="f1")
                    for ct in range(NCT):
                        nc.tensor.matmul(fp[:], w1_sb[:, oti, ct, :], h2t[:, ct, :],
                                         start=(ct == 0), stop=(ct == NCT - 1))
                    nc.scalar.activation(h3t[:, ot, :], fp[:], AF.Gelu,
                                         bias=b1_pp[:, ot:ot + 1])

                # fc2 + bias + residual -> out
                for oc in range(2):
                    f2t = [f2s.tile([P, 512], f32, tag="f2", name=f"f2_{oc}_{i}")
                           for i in range(NTC)]
                    for ct in range(HIDDEN // P):
                        w2t = w2p.tile([P, 512], bf16, tag="w2t")
                        nc.sync.dma_start(
                            w2t[:], w2T_d[ct * P:(ct + 1) * P,
                                          oc * 512:(oc + 1) * 512])
                        for ts in range(NTC):
                            nc.tensor.matmul(f2t[ts][:],
                                             h3t[:, ct, ts * P:(ts + 1) * P],
                                             w2t[:], start=(ct == 0),
                                             stop=(ct == HIDDEN // P - 1))
                    for ts in range(NTC):
                        t1 = mlv.tile([P, 512], f32, tag="t12")
                        nc.vector.tensor_tensor(t1[:], f2t[ts][:],
                                                b2_bc[:, oc * 512:(oc + 1) * 512],
                                                ALU.add)
                        t2 = mlv.tile([P, 512], f32, tag="t22")
                        nc.vector.tensor_tensor(t2[:], t1[:],
                                                res1[:, ts, oc * 512:(oc + 1) * 512],
                                                ALU.add)
                        nc.sync.dma_start(
                            out_d[ts * P:(ts + 1) * P, oc * 512:(oc + 1) * 512],
                            t2[:])

            w1_pool[0][0].release()
            w1b.release()

    nc.compile()
    return nc


def _get_program():
    global _PROG
    if _PROG is None:
        _PROG = _build_program()
    return _PROG


def _pack_cols(wT):
    """[C, O] -> [O//128, 128(p), C//128(k), 128(o)] so each o-tile DMA is contiguous."""
    C, O = wT.shape
    # out[ot, p, k, o] = wT[k*128+p, ot*128+o]
    return np.ascontiguousarray(
        wT.reshape(C // P, P, O // P, P).transpose(2, 1, 0, 3))


def _host_prep(x, ln1_g, ln1_b, w_qkv, w_proj, b_proj, ln2_g, ln2_b,
               w_fc1, b_fc1, w_fc2, b_fc2):
    """Per-core input dicts. Pure layout/weight-folding work (no activation math)."""
    f = np.float32
    bf = ml_dtypes.bfloat16
    x = np.asarray(x, f)
    g1 = np.asarray(ln1_g, f); b1 = np.asarray(ln1_b, f)
    g2 = np.asarray(ln2_g, f); b2 = np.asarray(ln2_b, f)
    w_qkv = np.asarray(w_qkv, f); w_proj = np.asarray(w_proj, f)
    w_fc1 = np.asarray(w_fc1, f); w_fc2 = np.asarray(w_fc2, f)
    b_proj = np.asarray(b_proj, f); b_fc1 = np.asarray(b_fc1, f)
    b_fc2 = np.asarray(b_fc2, f)

    wq, wk, wv = w_qkv[0:DIM], w_qkv[DIM:2 * DIM], w_qkv[2 * DIM:3 * DIM]
    wqg = (wq * g1[None, :]).T   # [C(f), O] feature-major, LN gain folded
    wkg = (wk * g1[None, :]).T
    wvg = (wv * g1[None, :]).T
    bq = wq @ b1; bk = wk @ b1; bv = wv @ b1

    wkq = np.concatenate([_pack_cols(wkg), _pack_cols(wqg)], axis=3)
    wkqc = np.stack([
        np.concatenate([wkg.sum(axis=0).reshape(NCT, P),
                        wqg.sum(axis=0).reshape(NCT, P)], axis=1),
        np.concatenate([bk.reshape(NCT, P), bq.reshape(NCT, P)], axis=1),
    ], axis=0)  # [2, 8, 256]

    sel = np.zeros((16, NCT, P), f)
    for j in range(NCT):
        sel[2 * j, j, 0:64] = 1.0
        sel[2 * j + 1, j, 64:128] = 1.0

    bp_total = b_proj + w_proj @ bv
    shared = {
        "wv_d": np.ascontiguousarray(wvg.astype(bf)),
        "wv1_d": np.ascontiguousarray(wvg.sum(axis=0).reshape(1, DIM).astype(bf)),
        "wkq_d": np.ascontiguousarray(wkq.astype(bf)),
        "wkqc_d": np.ascontiguousarray(wkqc.astype(bf)),
        "selb_d": np.ascontiguousarray(sel.astype(bf)),
        "wp_d": np.ascontiguousarray(
            w_proj.T.reshape(NCT, P, DIM).transpose(1, 0, 2).astype(bf)),
        "bpbc_d": np.ascontiguousarray(np.broadcast_to(bp_total, (P, DIM)).astype(f)),
        "w1h_d": np.ascontiguousarray(_pack_cols((w_fc1 * g2[None, :]).T).astype(bf)),
        "b1pp_d": np.ascontiguousarray(
            (b_fc1 + w_fc1 @ b2).reshape(HIDDEN // P, P).T.astype(f)),
        "w2T_d": np.ascontiguousarray(w_fc2.T.astype(bf)),
        "b2bc_d": np.ascontiguousarray(np.broadcast_to(b_fc2, (P, DIM)).astype(f)),
    }
    in_maps = []
    for core in range(8):
        b, q = core // 4, core % 4
        xroll = np.roll(x[b], -CH * q, axis=0)
        m = dict(shared)
        m["xtm_d"] = np.ascontiguousarray(xroll)
        xb = xroll.astype(bf)
        m["xsb_d"] = np.ascontiguousarray(xb)
        m["xTb_d"] = np.ascontiguousarray(xb.T)
        in_maps.append(m)
    return in_maps


def kernel(**inputs) -> np.ndarray:
    _setup_env()
    from concourse import bass_utils

    nc = _get_program()
    in_maps = _host_prep(**inputs)
    run_kwargs = {}
    if os.environ.get("BASS_PROFILE"):
        import tempfile
        run_kwargs = dict(trace=True, tmpdir=tempfile.mkdtemp(prefix="blk_prof"))
    res = bass_utils.run_bass_kernel_spmd(nc, in_maps, core_ids=list(range(8)),
                                          **run_kwargs)
    kernel.last_result = res
    out = np.empty((2, T, DIM), np.float32)
    for core in range(8):
        b, q = core // 4, core % 4
        out[b, CH * q:CH * (q + 1), :] = res.results[core]["out_d"]
    return out


# revision 12
# speedup vs baseline: 1.4757x; 1.0120x over previous
"""Trainium2 Bass kernel for a pre-norm transformer block (nn_Block_74766790689102).

v2 strategy (8 NeuronCores, zero-communication SPMD):
  core c handles batch b=c//4, query chunk q=c%4 (512 of 2048 tokens); inputs
  are host-rotated so each core's chunk sits at token positions 0:512 -> one
  identical SPMD program for all 8 cores. Each core redundantly computes K/V
  for its whole batch (attention needs all keys).

  Changes vs v1 (716us):
  - bf16 operands for every matmul (fp32 PSUM accumulation): halves weight
    DMA and enables fast weight loads on the PE.
  - Normalized x is never materialized. QKV matmuls consume raw x^T plus a
    rank-2 correction row ((-mu, sdv) x (col-sums, bias)) folded into the
    contraction, then a per-token rstd scale on the outputs. Kills the
    gpsimd/vector normalize traffic and one full 8MB x reload.
  - K, exp'd scores, attention outputs, res1 all stay in SBUF (no DRAM
    roundtrips).
  - K production for head-pair j+1, scores/exp for j, and PV for j-1 are
    interleaved in one PE stream so the ~134us of scalar-engine exp hides
    behind PE work.
  - Softmax denominators collect into one [16,512] tile; a single
    reciprocal_approx_fast + 8 PE broadcasts replace 16 serial [1,512] DVE
    reciprocals (was 52us).
  - V/proj biases fold into the proj bias on the host (softmax rows sum to 1).
  - w1 is fully resident in SBUF before FC1 starts; w2 streams as bf16.
"""

import os
import sys
import types

import numpy as np
import ml_dtypes

DIM = 1024
HEADS = 16
HD = 64
HIDDEN = 4096
T = 2048          # tokens per batch
CH = 512          # chunk tokens per core
SCALE = HD ** -0.5
EPS = 1e-5
NCT = DIM // 128  # 8 feature tiles
NTC = T // 512    # 4 token chunks
NTT = T // 128    # 16 token tiles
P = 128

_ENV_READY = False
_PROG = None


def _setup_env():
    global _ENV_READY
    if _ENV_READY:
        return
    if "/opt/trn_rl_repo" not in sys.path:
        sys.path.insert(0, "/opt/trn_rl_repo")
    # NTFF profile hook shim (the RL container's antenv lacks axon_hooks).
    try:
        import antenv
        if "antenv.axon_hooks" not in sys.modules:
            mod = types.ModuleType("antenv.axon_hooks")
            mod._hook = None
            mod.set_axon_ntff_profile_hook = lambda h: setattr(mod, "_hook", h)
            mod.get_axon_ntff_profile_hook = lambda: mod._hook
            sys.modules["antenv.axon_hooks"] = mod
            antenv.axon_hooks = mod
        if os.environ.get("BASS_PROFILE"):
            from trn_agent_boot.trn_boot import _ntff_profile_via_ctypes
            sys.modules["antenv.axon_hooks"].set_axon_ntff_profile_hook(
                _ntff_profile_via_ctypes("/opt/axon/libaxon_pjrt.so"))
    except Exception:
        pass
    _ENV_READY = True


def _build_program():
    """Build + compile the single-core Bass program (same for all 8 cores)."""
    _setup_env()
    import concourse.bacc as bacc
    import concourse.tile as tile
    import concourse.mybir as mybir
    from concourse.masks import make_identity

    dt = mybir.dt
    AF = mybir.ActivationFunctionType
    ALU = mybir.AluOpType
    f32, bf16 = dt.float32, dt.bfloat16

    nc = bacc.Bacc("TRN2", target_bir_lowering=False, debug=False, num_devices=8)

    # ---- I/O ----
    xtm_d = nc.dram_tensor("xtm_d", [T, DIM], f32, kind="ExternalInput").ap()
    xsb_d = nc.dram_tensor("xsb_d", [T, DIM], bf16, kind="ExternalInput").ap()
    xTb_d = nc.dram_tensor("xTb_d", [DIM, T], bf16, kind="ExternalInput").ap()
    wv_d = nc.dram_tensor("wv_d", [DIM, DIM], bf16, kind="ExternalInput").ap()
    wv1_d = nc.dram_tensor("wv1_d", [1, DIM], bf16, kind="ExternalInput").ap()
    wkq_d = nc.dram_tensor("wkq_d", [NCT, P, NCT, 256], bf16, kind="ExternalInput").ap()
    wkqc_d = nc.dram_tensor("wkqc_d", [2, NCT, 256], bf16, kind="ExternalInput").ap()
    selb_d = nc.dram_tensor("selb_d", [16, NCT, P], bf16, kind="ExternalInput").ap()
    wp_d = nc.dram_tensor("wp_d", [P, NCT, DIM], bf16, kind="ExternalInput").ap()
    bpbc_d = nc.dram_tensor("bpbc_d", [P, DIM], f32, kind="ExternalInput").ap()
    w1h_d = nc.dram_tensor("w1h_d", [HIDDEN // P, P, NCT, P], bf16, kind="ExternalInput").ap()
    b1pp_d = nc.dram_tensor("b1pp_d", [P, HIDDEN // P], f32, kind="ExternalInput").ap()
    w2T_d = nc.dram_tensor("w2T_d", [HIDDEN, DIM], bf16, kind="ExternalInput").ap()
    b2bc_d = nc.dram_tensor("b2bc_d", [P, DIM], f32, kind="ExternalInput").ap()
    out_d = nc.dram_tensor("out_d", [CH, DIM], f32, kind="ExternalOutput").ap()

    with tile.TileContext(nc) as tc:
        with tc.tile_pool(name="cst", bufs=1) as cst, \
             tc.tile_pool(name="resp", bufs=1) as resp:

            # ---------------- constants ----------------
            idf = cst.tile([P, P], f32)
            make_identity(nc, idf[:])
            idb = cst.tile([P, P], bf16)
            nc.vector.tensor_copy(idb[:], idf[:])
            ones1b = cst.tile([1, P], bf16)
            nc.vector.memset(ones1b[:], 1.0)
            eps_t = cst.tile([P, 1], f32)
            nc.vector.memset(eps_t[:], EPS)
            # head-pair selection matrices for the denominator broadcast
            sel = cst.tile([16, NCT, P], bf16)
            nc.sync.dma_start(sel[:], selb_d[:])
            corr2 = cst.tile([2, T], bf16)     # rows: -mu, sdv (per token)
            rsr = cst.tile([1, T], bf16)       # rstd row (per token)
            rs_cols = cst.tile([P, NTT], f32)  # rstd, token-partition layout
            dnall = cst.tile([16, 512], f32)   # softmax denominators per head
            rc_f = cst.tile([16, 512], f32)
            rc_bf = cst.tile([16, 512], bf16)
            b1_pp = cst.tile([P, HIDDEN // P], f32)
            nc.sync.dma_start(b1_pp[:], b1pp_d[:])

            res1 = resp.tile([P, NTC, DIM], f32)   # attn residual stream
            b2_bc = resp.tile([P, DIM], f32)
            nc.sync.dma_start(b2_bc[:], b2bc_d[:])

            # attention-lifetime SBUF (released after proj)
            atn = tc.alloc_tile_pool(name="atn", bufs=1)
            vsb = atn.tile([P, NTT, HEADS, 65], bf16)   # V-hat + ones col 64
            q_sb = atn.tile([P, NCT, 512], bf16)
            k_sb = atn.tile([P, NCT, T], bf16)
            osb = atn.tile([P, NCT, 512], bf16)         # per-pair attn out
            rb_sb = atn.tile([P, NTC, 512], bf16)       # rstd broadcast

            nc.vector.memset(
                vsb[:, :, :, 64:65].rearrange("p a b c -> p (a b c)"), 1.0)

            # raw x^T + K/Q weights (released before FC1 weight prefetch)
            xTp = tc.alloc_tile_pool(name="xTp", bufs=1, side="right")
            xT_sb = xTp.tile([P, NCT, T], bf16)
            kqw = tc.alloc_tile_pool(name="kqw", bufs=2, side="right")
            wkqc_sb = kqw.tile([2, NCT, 256], bf16, tag="wkqc", bufs=1)
            nc.sync.dma_start(wkqc_sb[:], wkqc_d[:])

            kqp = tc.alloc_tile_pool(name="kqp", bufs=1, space="PSUM")
            wkq_tiles = {}

            def load_wkq(j):
                w = kqw.tile([P, NCT, 256], bf16, tag="wkq", name=f"wkq{j}")
                nc.sync.dma_start(w[:], wkq_d[j])
                wkq_tiles[j] = w

            def emit_k_chunk(j, tcn):
                w = wkq_tiles[j]
                kp = kqp.tile([P, 512], f32, tag="kq", name=f"kp{j}_{tcn}")
                sl = slice(tcn * 512, (tcn + 1) * 512)
                for k in range(NCT):
                    nc.tensor.matmul(kp[:], w[:, k, 0:P], xT_sb[:, k, sl],
                                     start=(k == 0), stop=False)
                nc.tensor.matmul(kp[:], wkqc_sb[:, j, 0:P], corr2[0:2, sl],
                                 start=False, stop=True)
                nc.vector.tensor_tensor(k_sb[:, j, sl], kp[:], rb_sb[:, tcn, :],
                                        ALU.mult)

            def emit_q(j):
                w = wkq_tiles.pop(j)
                qp = kqp.tile([P, 512], f32, tag="kq", name=f"qp{j}")
                for k in range(NCT):
                    nc.tensor.matmul(qp[:], w[:, k, P:256], xT_sb[:, k, 0:512],
                                     start=(k == 0), stop=False)
                nc.tensor.matmul(qp[:], wkqc_sb[:, j, P:256], corr2[0:2, 0:512],
                                 start=False, stop=True)
                nc.vector.tensor_tensor(q_sb[:, j, :], qp[:], rb_sb[:, 0, :],
                                        ALU.mult)

            load_wkq(0)
            load_wkq(1)

            # ---------------- Phase A: LN1 stats + V-hat ----------------
            with tc.tile_pool(name="xsp", bufs=2) as xsp, \
                 tc.tile_pool(name="stp", bufs=2) as stp, \
                 tc.tile_pool(name="wvp", bufs=1) as wvp, \
                 tc.tile_pool(name="aps", bufs=1, space="PSUM") as aps, \
                 tc.tile_pool(name="rbp", bufs=1, space="PSUM") as rbp, \
                 tc.tile_pool(name="vps", bufs=3, space="PSUM") as vps:
                wv_sb = wvp.tile([P, NCT, DIM], bf16)
                wv1_sb = wvp.tile([1, DIM], bf16)
                nc.sync.dma_start(wv1_sb[:], wv1_d[:])
                # per-k interleave so the V-hat k-chain starts as soon as the
                # first contraction tile lands instead of after the full 6MB
                for k in range(NCT):
                    nc.sync.dma_start(wv_sb[:, k, :],
                                      wv_d[k * P:(k + 1) * P, :])
                    nc.sync.dma_start(xT_sb[:, k, :], xTb_d[k * P:(k + 1) * P, :])
                for s in range(NTT):
                    xs = xsp.tile([P, DIM], bf16, tag="xs")
                    nc.sync.dma_start(xs[:], xsb_d[s * P:(s + 1) * P, :])
                    stats = stp.tile([P, 2, 6], f32, tag="bst")
                    for g in range(2):
                        nc.vector.bn_stats(stats[:, g, :], xs[:, g * 512:(g + 1) * 512])
                    mv = stp.tile([P, 2], f32, tag="mv")
                    nc.vector.bn_aggr(mv[:], stats[:])
                    stk = stp.tile([P, 2], f32, tag="stk")
                    nc.vector.tensor_scalar(stk[:, 0:1], mv[:, 0:1], -1.0, None, ALU.mult)
                    nc.scalar.activation(stk[:, 1:2], mv[:, 1:2], AF.Sqrt, bias=eps_t[:])
                    nc.vector.reciprocal(rs_cols[:, s:s + 1], stk[:, 1:2])
                    pst = aps.tile([2, P], f32, tag="pst")
                    nc.tensor.transpose(pst[:], stk[:], idf[:])
                    nc.vector.tensor_copy(corr2[:, s * P:(s + 1) * P], pst[:])
                    pst1 = aps.tile([1, P], f32, tag="pst1")
                    nc.tensor.transpose(pst1[:], rs_cols[:, s:s + 1], idf[:])
                    nc.vector.tensor_copy(rsr[:, s * P:(s + 1) * P], pst1[:])
                    # V-hat for token tile s (raw x + rank-1 mu correction)
                    for oc in range(2):
                        vp = vps.tile([P, 512], f32, tag="vp")
                        for k in range(NCT):
                            nc.tensor.matmul(vp[:], xT_sb[:, k, s * P:(s + 1) * P],
                                             wv_sb[:, k, oc * 512:(oc + 1) * 512],
                                             start=(k == 0), stop=False)
                        nc.tensor.matmul(vp[:], corr2[0:1, s * P:(s + 1) * P],
                                         wv1_sb[0:1, oc * 512:(oc + 1) * 512],
                                         start=False, stop=True)
                        nc.vector.tensor_scalar(
                            vsb[:, s, oc * 8:(oc + 1) * 8, 0:64],
                            vp[:].rearrange("p (h d) -> p h d", d=64),
                            rs_cols[:, s:s + 1], None, ALU.mult)
                    if s % 4 == 3:
                        tcn = s // 4
                        rb = rbp.tile([P, 512], f32, tag="rb")
                        nc.tensor.matmul(rb[:], ones1b[:],
                                         rsr[0:1, tcn * 512:(tcn + 1) * 512],
                                         start=True, stop=True)
                        nc.vector.tensor_copy(rb_sb[:, tcn, :], rb[:])
                        # hide the first two K head-tiles + Q(0) under V-hat
                        emit_k_chunk(0, tcn)
                        emit_k_chunk(1, tcn)
                        if s == NTT - 1:
                            emit_q(0)
                            load_wkq(2)

            # ---------------- Phase B+C: scores/exp + PV + remaining K/Q ----------------
            psb_slots = {}
            w1_pool = []   # filled mid-phase once xT frees

            with tc.tile_pool(name="psbp", bufs=14) as psbp, \
                 tc.tile_pool(name="tvec", bufs=2) as tvec, \
                 tc.tile_pool(name="spp", bufs=2, space="PSUM") as spp, \
                 tc.tile_pool(name="pvp", bufs=3, space="PSUM") as pvp:

                def emit_s(j, kt):
                    sp = spp.tile([P, 2, 512], f32, tag="sp", name=f"sp{j}_{kt}")
                    ksl = slice(kt * P, (kt + 1) * P)
                    nc.tensor.matmul(sp[:, 0, :], k_sb[0:64, j, ksl],
                                     q_sb[0:64, j, :], start=True, stop=True)
                    nc.tensor.matmul(sp[:, 1, :], k_sb[64:128, j, ksl],
                                     q_sb[64:128, j, :], start=True, stop=True)
                    slot = psbp.tile([P, 2, 512], bf16, tag="psb",
                                     name=f"psb{j}_{kt}")
                    nc.scalar.activation(slot[:], sp[:], AF.Exp, scale=SCALE)
                    psb_slots[(j, kt)] = slot

                def emit_pv_kt(j, kt, pvA, pvB):
                    slot = psb_slots.pop((j, kt))
                    nc.tensor.matmul(pvA[:], vsb[:, kt, 2 * j, :], slot[:, 0, :],
                                     start=(kt == 0), stop=(kt == NTT - 1))
                    nc.tensor.matmul(pvB[:], vsb[:, kt, 2 * j + 1, :], slot[:, 1, :],
                                     start=(kt == 0), stop=(kt == NTT - 1))

                def emit_tail(h, pv):
                    dtmp = tvec.tile([65, 512], f32, tag="dtmp", name=f"dt{h}")
                    nc.scalar.activation(dtmp[64:65, :], pv[64:65, :], AF.Copy)
                    nc.sync.dma_start(dnall[h:h + 1, :], dtmp[64:65, :])
                    if h % 2 == 0:
                        nc.vector.tensor_copy(osb[0:64, h // 2, :], pv[0:64, :])
                    else:
                        ot = tvec.tile([64, 512], bf16, tag="ot", name=f"ot{h}")
                        nc.vector.tensor_copy(ot[:], pv[0:64, :])
                        nc.sync.dma_start(osb[64:128, h // 2, :], ot[:])

                pvA = pvB = None
                for j in range(NCT):
                    if j == NCT - 1:
                        # x^T and K/Q weights are dead: free them; prefetch
                        # proj weights and the first w1 slice under the last
                        # scores/PV round.
                        kqw.release()
                        xTp.release()
                        w1a = tc.alloc_tile_pool(name="w1a", bufs=1,
                                                 side="right")
                        w1_sb_a = w1a.tile([P, 8, NCT, P], bf16)
                        for og in range(2):
                            nc.sync.dma_start(
                                w1_sb_a[:, og * 4:(og + 1) * 4, :, :],
                                w1h_d[og * 4:(og + 1) * 4].rearrange(
                                    "a p b c -> p a b c"))
                        w1_pool.append((w1a, w1_sb_a))
                        wpx = tc.alloc_tile_pool(name="wpx", bufs=1,
                                                 side="right")
                        wp_sb = wpx.tile([P, NCT, DIM], bf16)
                        nc.sync.dma_start(wp_sb[:], wp_d[:])
                        bp_bc = wpx.tile([P, DIM], f32)
                        nc.sync.dma_start(bp_bc[:], bpbc_d[:])
                    if j > 0:
                        pvA = pvp.tile([65, 512], f32, tag="pv", name=f"pvA{j - 1}")
                        pvB = pvp.tile([65, 512], f32, tag="pv", name=f"pvB{j - 1}")
                    for kt in range(NTT):
                        emit_s(j, kt)
                        if kt in (1, 5, 9, 13) and j + 2 < NCT:
                            emit_k_chunk(j + 2, kt // 4)
                        elif kt == 14 and j + 1 < NCT:
                            emit_q(j + 1)
                            if j + 3 < NCT:
                                load_wkq(j + 3)
                        if j > 0:
                            emit_pv_kt(j - 1, kt, pvA, pvB)
                    if j > 0:
                        emit_tail(2 * (j - 1), pvA)
                        emit_tail(2 * (j - 1) + 1, pvB)
                # last pair
                pvA = pvp.tile([65, 512], f32, tag="pv", name="pvA7")
                pvB = pvp.tile([65, 512], f32, tag="pv", name="pvB7")
                for kt in range(NTT):
                    emit_pv_kt(NCT - 1, kt, pvA, pvB)
                emit_tail(2 * (NCT - 1), pvA)
                emit_tail(2 * (NCT - 1) + 1, pvB)

            # ---------------- Phase D: normalize o, proj + residual ----------------
            kqp.release()
            with tc.tile_pool(name="dvec", bufs=2) as dvec, \
                 tc.tile_pool(name="xrp", bufs=4) as xrp, \
                 tc.tile_pool(name="bcp", bufs=2, space="PSUM") as bcp, \
                 tc.tile_pool(name="pjp", bufs=2, space="PSUM") as pjp:
                xres_t = []
                for ts in range(NTC):
                    xres = xrp.tile([P, DIM], f32, tag="xres", name=f"xr{ts}")
                    nc.sync.dma_start(xres[:], xtm_d[ts * P:(ts + 1) * P, :])
                    xres_t.append(xres)
                nc.vector.reciprocal_approx_fast(rc_f[:], dnall[:])
                nc.vector.tensor_copy(rc_bf[:], rc_f[:])
                for j in range(NCT):
                    bc = bcp.tile([P, 512], f32, tag="bc", name=f"bc{j}")
                    nc.tensor.matmul(bc[:], sel[:, j, :], rc_bf[:],
                                     start=True, stop=True)
                    rcb = dvec.tile([P, 512], bf16, tag="rcb", name=f"rcb{j}")
                    nc.scalar.activation(rcb[:], bc[:], AF.Copy)
                    nc.vector.tensor_tensor(osb[:, j, :], osb[:, j, :], rcb[:],
                                            ALU.mult)
                for ts in range(NTC):
                    xres = xres_t[ts]
                    for oc in range(2):
                        pj = pjp.tile([P, 512], f32, tag="pj")
                        for j in range(NCT):
                            nc.tensor.matmul(pj[:], osb[:, j, ts * P:(ts + 1) * P],
                                             wp_sb[:, j, oc * 512:(oc + 1) * 512],
                                             start=(j == 0), stop=(j == NCT - 1))
                        t1 = dvec.tile([P, 512], f32, tag="t1")
                        nc.vector.tensor_tensor(t1[:], pj[:],
                                                xres[:, oc * 512:(oc + 1) * 512],
                                                ALU.add)
                        nc.vector.tensor_tensor(
                            res1[:, ts, oc * 512:(oc + 1) * 512], t1[:],
                            bp_bc[:, oc * 512:(oc + 1) * 512], ALU.add)

            atn.release()
            wpx.release()

            # rest of w1 (space freed by attention buffers)
            w1b = tc.alloc_tile_pool(name="w1b", bufs=1)
            w1_sb_b = w1b.tile([P, 24, NCT, P], bf16)
            for og in range(6):
                nc.sync.dma_start(
                    w1_sb_b[:, og * 4:(og + 1) * 4, :, :],
                    w1h_d[8 + og * 4:8 + (og + 1) * 4].rearrange(
                        "a p b c -> p a b c"))

            # ---------------- Phase E: LN2 + MLP ----------------
            with tc.tile_pool(name="st2", bufs=2) as st2, \
                 tc.tile_pool(name="h2p", bufs=2) as h2p, \
                 tc.tile_pool(name="h2tp", bufs=1) as h2tp, \
                 tc.tile_pool(name="h3p", bufs=1) as h3p, \
                 tc.tile_pool(name="w2p", bufs=6) as w2p, \
                 tc.tile_pool(name="mlv", bufs=2) as mlv, \
                 tc.tile_pool(name="tps", bufs=2, space="PSUM") as tps, \
                 tc.tile_pool(name="f1s", bufs=2, space="PSUM") as f1s, \
                 tc.tile_pool(name="f2s", bufs=4, space="PSUM") as f2s:
                h2t = h2tp.tile([P, NCT, 512], bf16)
                h3t = h3p.tile([P, HIDDEN // P, 512], bf16)

                for ts in range(NTC):
                    stats2 = st2.tile([P, 2, 6], f32, tag="bst2")
                    for g in range(2):
                        nc.vector.bn_stats(stats2[:, g, :],
                                           res1[:, ts, g * 512:(g + 1) * 512])
                    mv2 = st2.tile([P, 2], f32, tag="mv2")
                    nc.vector.bn_aggr(mv2[:], stats2[:])
                    sdv2 = st2.tile([P, 1], f32, tag="sdv2")
                    nc.scalar.activation(sdv2[:], mv2[:, 1:2], AF.Sqrt, bias=eps_t[:])
                    rs2 = st2.tile([P, 1], f32, tag="rs2")
                    nc.vector.reciprocal(rs2[:], sdv2[:])
                    nmu = st2.tile([P, 1], f32, tag="nmu")
                    nc.vector.tensor_tensor(nmu[:], mv2[:, 0:1], rs2[:], ALU.mult)
                    nc.vector.tensor_scalar(nmu[:], nmu[:], -1.0, None, ALU.mult)
                    h2 = h2p.tile([P, DIM], bf16, tag="h2")
                    nc.scalar.activation(h2[:], res1[:, ts, :], AF.Identity,
                                         bias=nmu[:], scale=rs2[:])
                    for ct in range(NCT):
                        tp = tps.tile([P, P], bf16, tag="tp")
                        nc.tensor.transpose(tp[:], h2[:, ct * P:(ct + 1) * P], idb[:])
                        nc.vector.tensor_copy(h2t[:, ct, ts * P:(ts + 1) * P], tp[:])

                # fc1 + gelu -> h3t (hidden-major)
                for ot in range(HIDDEN // P):
                    w1_sb = w1_pool[0][1] if ot < 8 else w1_sb_b
                    oti = ot if ot < 8 else ot - 8
                    fp = f1s.tile([P, 512], f32, tag="f1")
                    for ct in range(NCT):
                        nc.tensor.matmul(fp[:], w1_sb[:, oti, ct, :], h2t[:, ct, :],
                                         start=(ct == 0), stop=(ct == NCT - 1))
                    nc.scalar.activation(h3t[:, ot, :], fp[:], AF.Gelu,
                                         bias=b1_pp[:, ot:ot + 1])

                # fc2 + bias + residual -> out
                for oc in range(2):
                    f2t = [f2s.tile([P, 512], f32, tag="f2", name=f"f2_{oc}_{i}")
                           for i in range(NTC)]
                    for ct in range(HIDDEN // P):
                        w2t = w2p.tile([P, 512], bf16, tag="w2t")
                        nc.sync.dma_start(
                            w2t[:], w2T_d[ct * P:(ct + 1) * P,
                                          oc * 512:(oc + 1) * 512])
                        for ts in range(NTC):
                            nc.tensor.matmul(f2t[ts][:],
                                             h3t[:, ct, ts * P:(ts + 1) * P],
                                             w2t[:], start=(ct == 0),
                                             stop=(ct == HIDDEN // P - 1))
                    for ts in range(NTC):
                        t1 = mlv.tile([P, 512], f32, tag="t12")
                        nc.vector.tensor_tensor(t1[:], f2t[ts][:],
                                                b2_bc[:, oc * 512:(oc + 1) * 512],
                                                ALU.add)
                        t2 = mlv.tile([P, 512], f32, tag="t22")
                        nc.vector.tensor_tensor(t2[:], t1[:],
                                                res1[:, ts, oc * 512:(oc + 1) * 512],
                                                ALU.add)
                        nc.sync.dma_start(
                            out_d[ts * P:(ts + 1) * P, oc * 512:(oc + 1) * 512],
                            t2[:])

            w1_pool[0][0].release()
            w1b.release()

    nc.compile()
    return nc


def _get_program():
    global _PROG
    if _PROG is None:
        _PROG = _build_program()
    return _PROG


def _pack_cols(wT):
    """[C, O] -> [O//128, 128(p), C//128(k), 128(o)] so each o-tile DMA is contiguous."""
    C, O = wT.shape
    # out[ot, p, k, o] = wT[k*128+p, ot*128+o]
    return np.ascontiguousarray(
        wT.reshape(C // P, P, O // P, P).transpose(2, 1, 0, 3))


def _host_prep(x, ln1_g, ln1_b, w_qkv, w_proj, b_proj, ln2_g, ln2_b,
               w_fc1, b_fc1, w_fc2, b_fc2):
    """Per-core input dicts. Pure layout/weight-folding work (no activation math)."""
    f = np.float32
    bf = ml_dtypes.bfloat16
    x = np.asarray(x, f)
    g1 = np.asarray(ln1_g, f); b1 = np.asarray(ln1_b, f)
    g2 = np.asarray(ln2_g, f); b2 = np.asarray(ln2_b, f)
    w_qkv = np.asarray(w_qkv, f); w_proj = np.asarray(w_proj, f)
    w_fc1 = np.asarray(w_fc1, f); w_fc2 = np.asarray(w_fc2, f)
    b_proj = np.asarray(b_proj, f); b_fc1 = np.asarray(b_fc1, f)
    b_fc2 = np.asarray(b_fc2, f)

    wq, wk, wv = w_qkv[0:DIM], w_qkv[DIM:2 * DIM], w_qkv[2 * DIM:3 * DIM]
    wqg = (wq * g1[None, :]).T   # [C(f), O] feature-major, LN gain folded
    wkg = (wk * g1[None, :]).T
    wvg = (wv * g1[None, :]).T
    bq = wq @ b1; bk = wk @ b1; bv = wv @ b1

    wkq = np.concatenate([_pack_cols(wkg), _pack_cols(wqg)], axis=3)
    wkqc = np.stack([
        np.concatenate([wkg.sum(axis=0).reshape(NCT, P),
                        wqg.sum(axis=0).reshape(NCT, P)], axis=1),
        np.concatenate([bk.reshape(NCT, P), bq.reshape(NCT, P)], axis=1),
    ], axis=0)  # [2, 8, 256]

    sel = np.zeros((16, NCT, P), f)
    for j in range(NCT):
        sel[2 * j, j, 0:64] = 1.0
        sel[2 * j + 1, j, 64:128] = 1.0

    bp_total = b_proj + w_proj @ bv
    shared = {
        "wv_d": np.ascontiguousarray(wvg.astype(bf)),
        "wv1_d": np.ascontiguousarray(wvg.sum(axis=0).reshape(1, DIM).astype(bf)),
        "wkq_d": np.ascontiguousarray(wkq.astype(bf)),
        "wkqc_d": np.ascontiguousarray(wkqc.astype(bf)),
        "selb_d": np.ascontiguousarray(sel.astype(bf)),
        "wp_d": np.ascontiguousarray(
            w_proj.T.reshape(NCT, P, DIM).transpose(1, 0, 2).astype(bf)),
        "bpbc_d": np.ascontiguousarray(np.broadcast_to(bp_total, (P, DIM)).astype(f)),
        "w1h_d": np.ascontiguousarray(_pack_cols((w_fc1 * g2[None, :]).T).astype(bf)),
        "b1pp_d": np.ascontiguousarray(
            (b_fc1 + w_fc1 @ b2).reshape(HIDDEN // P, P).T.astype(f)),
        "w2T_d": np.ascontiguousarray(w_fc2.T.astype(bf)),
        "b2bc_d": np.ascontiguousarray(np.broadcast_to(b_fc2, (P, DIM)).astype(f)),
    }
    in_maps = []
    for core in range(8):
        b, q = core // 4, core % 4
        xroll = np.roll(x[b], -CH * q, axis=0)
        m = dict(shared)
        m["xtm_d"] = np.ascontiguousarray(xroll)
        xb = xroll.astype(bf)
        m["xsb_d"] = np.ascontiguousarray(xb)
        m["xTb_d"] = np.ascontiguousarray(xb.T)
        in_maps.append(m)
    return in_maps


def kernel(**inputs) -> np.ndarray:
    _setup_env()
    from concourse import bass_utils

    nc = _get_program()
    in_maps = _host_prep(**inputs)
    run_kwargs = {}
    if os.environ.get("BASS_PROFILE"):
        import tempfile
        run_kwargs = dict(trace=True, tmpdir=tempfile.mkdtemp(prefix="blk_prof"))
    res = bass_utils.run_bass_kernel_spmd(nc, in_maps, core_ids=list(range(8)),
                                          **run_kwargs)
    kernel.last_result = res
    out = np.empty((2, T, DIM), np.float32)
    for core in range(8):
        b, q = core // 4, core % 4
        out[b, CH * q:CH * (q + 1), :] = res.results[core]["out_d"]
    return out
